# revision 11
# baseline (speedup 1.0000x reference)
"""Cross-attention MHA (B=2, S=2048, DIM=1024, H=16, DK=64) on 8 trn2 cores.

Sharding: core c -> batch b = c//4, head group g = c%4 (heads 4g..4g+3).
Each core computes its 4 heads' attention + output-projection partial
[S, DIM] in bf16; host sums the 4 partials per batch.

v2 design (vs v1 phase-serial):
  The scalar engine's exp stream (~147us: 128 tiles of [128,1024], 1
  elem/cycle/lane @1.2GHz) is the hard bottleneck; the PE's matmul work
  (~125us warm) must ride inside that envelope instead of serializing
  with it. One continuous emission stream:
    prologue: Q proj -> K proj (evictions on ACT, DMA-paced)
    4 attention segments (qh, pr) of 16 k-blocks each, software-pipelined
      scores (PE, auto row-tiled pairs) -> exp (ACT) -> mask-mul (DVE)
      -> AV accumulate (PE)
    V-projection matmuls interleaved into segment 0 (PE slack),
    output projection for qh=0 interleaved into segment 2,
    normalize fused into AV eviction: den copy + reciprocal_approx_fast
      + gpsimd partition_broadcast + one scalar_tensor_tensor per head.
  PSUM: scores tag 4 banks + av/vproj/po shared tag 4 banks.
  Mask DMAs go through the gpsimd ring so they don't queue behind x/w
  loads on the sync ring. Output is bf16 (host accumulates in f32).

Device layout (per core), transposed so no on-chip transposes needed:
  xd = dec_xs[b].T     [DIM, S]   bf16
  xe = enc_xs[b].T     [DIM, S]   bf16
  mt = mask[b].T       [S_k, S_q] bf16   (0.0 / 1.0)
  wq/wk/wv [128, DIM/128, 256] bf16  cols (head_local, dk); wq pre-scaled
  wo [128, 256/128, DIM] bf16  rows (head_local, dk)
"""

import sys

sys.path.insert(0, "/opt/trn_rl_repo")

import numpy as np
import ml_dtypes

import concourse.bass as bass
import concourse.mybir as mybir
import concourse.tile as tile
from concourse import bacc
from concourse.bass_utils import run_bass_kernel_spmd

B, S, DIM, H, DK = 2, 2048, 1024, 16, 64
HPC = 4  # heads per core
AD = HPC * DK  # 256 local attention dims
KB = S // 128  # 16 k-blocks
QH = 1024  # q half
NKC = DIM // 128  # 8 contraction chunks
BF = mybir.dt.bfloat16
F32 = mybir.dt.float32
bf16 = ml_dtypes.bfloat16

P2_BUFS = 10  # p-tile ring depth (covers lag-8 segments + cross-segment drain)


def build_program():
    nc = bacc.Bacc("TRN2", target_bir_lowering=False, debug=False, num_devices=8)

    xd = nc.dram_tensor("xd", [DIM, S], BF, kind="ExternalInput")
    xe = nc.dram_tensor("xe", [DIM, S], BF, kind="ExternalInput")
    mt = nc.dram_tensor("mt", [S, S], BF, kind="ExternalInput")
    wq = nc.dram_tensor("wq", [128, NKC, AD], BF, kind="ExternalInput")
    wk = nc.dram_tensor("wk", [128, NKC, AD], BF, kind="ExternalInput")
    wv = nc.dram_tensor("wv", [128, NKC, AD], BF, kind="ExternalInput")
    wo = nc.dram_tensor("wo", [128, AD // 128, DIM], BF, kind="ExternalInput")
    out = nc.dram_tensor("out", [S, DIM], BF, kind="ExternalOutput")

    with tile.TileContext(nc) as tc:
        build_tiles(tc, nc, xd, xe, mt, wq, wk, wv, wo, out)

    nc.compile()
    return nc


def build_tiles(tc, nc, xd, xe, mt, wq, wk, wv, wo, out):
    from contextlib import ExitStack

    Exp = mybir.ActivationFunctionType.Exp
    Mult = mybir.AluOpType.mult

    with ExitStack() as ctx:
        wpool = ctx.enter_context(tc.tile_pool(name="w", bufs=1))
        qkpool = ctx.enter_context(tc.tile_pool(name="qk", bufs=1))
        vpool = ctx.enter_context(tc.tile_pool(name="v", bufs=1))
        attpool = ctx.enter_context(tc.tile_pool(name="att", bufs=1))
        mpool = ctx.enter_context(tc.tile_pool(name="m", bufs=1))
        xepool = ctx.enter_context(tc.tile_pool(name="xe", bufs=1))
        denpool = ctx.enter_context(tc.tile_pool(name="den", bufs=1))
        recpool = ctx.enter_context(tc.tile_pool(name="rec", bufs=1))
        rbpool = ctx.enter_context(tc.tile_pool(name="rb", bufs=2))
        obpool = ctx.enter_context(tc.tile_pool(name="ob", bufs=3))

        # ---- long-lived SBUF ----
        wq_sb = wpool.tile([128, NKC, AD], BF, tag="wq", name="wq_sb")
        wk_sb = wpool.tile([128, NKC, AD], BF, tag="wk", name="wk_sb")
        wv_sb = wpool.tile([128, NKC, AD], BF, tag="wv", name="wv_sb")
        wo_sb = wpool.tile([128, AD // 128, DIM], BF, tag="wo", name="wo_sb")
        qt_sb = [qkpool.tile([128, S], BF, tag=f"qt{m}", name=f"qt{m}") for m in range(2)]
        kt_sb = [qkpool.tile([128, S], BF, tag=f"kt{m}", name=f"kt{m}") for m in range(2)]
        # V with a ones column at index 64 (row 64 of AV psum = softmax denom)
        v_sb = vpool.tile([128, KB, HPC, 65], BF, tag="v", name="v_sb")
        att_q = {
            (pr, qh): attpool.tile([128, QH], BF, tag=f"att{pr}{qh}", name=f"att{pr}{qh}")
            for pr in range(2)
            for qh in range(2)
        }
        m_sb = mpool.tile([128, KB, S], BF, tag="mask", name="m_sb")
        xe_sb = xepool.tile([128, NKC, S], BF, tag="xe", name="xe_sb")
        den_sb = denpool.tile([1, 2 * QH], F32, tag="den", name="den_sb")
        rec_sb = recpool.tile([1, 2 * QH], F32, tag="rec", name="rec_sb")

        # ---- prologue DMAs: xd chunks + wq first (Q proj gates everything) ----
        # xd pool opened/closed around Q proj so its space can be reused later
        xts = {}
        xdpool = tc.alloc_tile_pool(name="xd", bufs=2)
        for kc in range(2):
            xt = xdpool.tile([128, S], BF, tag="xd", name="xd_t")
            nc.sync.dma_start(xt[:], xd.ap()[kc * 128 : (kc + 1) * 128, :])
            xts[kc] = xt
        nc.sync.dma_start(wq_sb[:], wq.ap())

        # warm the ACT exp table off the critical path
        nc.scalar.activation(den_sb[0:1, 0:16], den_sb[0:1, 16:32], Exp)

        # ---- Q projection (kc-outer, xd prefetch, xe DMA interleave) ----
        with tc.tile_pool(name="psQ", bufs=4, space="PSUM") as psQ:
            ps_q = [psQ.tile([128, QH], F32, tag="ps_q", name=f"psq{i}") for i in range(4)]
            for kc in range(NKC):
                xt = xts.pop(kc)
                if kc + 2 < NKC:
                    nxt = xdpool.tile([128, S], BF, tag="xd", name="xd_t")
                    nc.sync.dma_start(nxt[:], xd.ap()[(kc + 2) * 128 : (kc + 3) * 128, :])
                    xts[kc + 2] = nxt
                for m in range(2):
                    for qq in range(2):
                        for nb in range(2):
                            nc.tensor.matmul(
                                ps_q[m * 2 + qq][:, nb * 512 : (nb + 1) * 512],
                                lhsT=wq_sb[:, kc, m * 128 : (m + 1) * 128],
                                rhs=xt[:, qq * QH + nb * 512 : qq * QH + (nb + 1) * 512],
                                start=(kc == 0),
                                stop=(kc == NKC - 1),
                            )
                nc.sync.dma_start(xe_sb[:, kc, :], xe.ap()[kc * 128 : (kc + 1) * 128, :])
            nc.sync.dma_start(wk_sb[:], wk.ap())
            nc.sync.dma_start(wv_sb[:], wv.ap())
            # mask DMAs: gpsimd ring (issue off the sync queue), emitted after
            # the xe/w loads so their 8MB doesn't starve the prologue of HBM
            for kb in range(KB):
                nc.gpsimd.dma_start(m_sb[:, kb, :], mt.ap()[kb * 128 : (kb + 1) * 128, :])
            # evictions on ACT (idle in prologue); (m=0,qq=0) first: gates seg 0
            for m in range(2):
                for qq in range(2):
                    nc.scalar.copy(qt_sb[m][:, qq * QH : (qq + 1) * QH], ps_q[m * 2 + qq][:])
        xdpool.release()
        nc.sync.dma_start(wo_sb[:], wo.ap())

        # ---- K projection ----
        with tc.tile_pool(name="psK", bufs=4, space="PSUM") as psK:
            ps_k = [psK.tile([128, QH], F32, tag="ps_k", name=f"psk{i}") for i in range(4)]
            for kc in range(NKC):
                for m in range(2):
                    for qq in range(2):
                        for nb in range(2):
                            nc.tensor.matmul(
                                ps_k[m * 2 + qq][:, nb * 512 : (nb + 1) * 512],
                                lhsT=wk_sb[:, kc, m * 128 : (m + 1) * 128],
                                rhs=xe_sb[:, kc, qq * QH + nb * 512 : qq * QH + (nb + 1) * 512],
                                start=(kc == 0),
                                stop=(kc == NKC - 1),
                            )
            for m in range(2):
                for qq in range(2):
                    nc.scalar.copy(kt_sb[m][:, qq * QH : (qq + 1) * QH], ps_k[m * 2 + qq][:])

        # ones for the denominator column (before first AV accumulate)
        nc.vector.memset(v_sb[:, :, :, 64:65], 1.0)

        # ---- main stream: attention segments + interleaved vproj / oproj ----
        p2pool = tc.alloc_tile_pool(name="p2", bufs=P2_BUFS)
        with tc.tile_pool(name="psM", bufs=2, space="PSUM") as psM:

            ob_state = {"i": 0}

            def emit_po(qh, i):
                # output projection for q-block i of half qh (128 rows)
                po = psM.tile([128, DIM], F32, tag="av", name="po")
                for cc in range(2):
                    for nb in range(2):
                        nc.tensor.matmul(
                            po[:, nb * 512 : (nb + 1) * 512],
                            lhsT=att_q[(cc, qh)][:, i * 128 : (i + 1) * 128],
                            rhs=wo_sb[:, cc, nb * 512 : (nb + 1) * 512],
                            start=(cc == 0),
                            stop=(cc == 1),
                        )
                ob = obpool.tile([128, DIM], BF, tag="ob", name="ob")
                # qh=0 runs inside the exp stretch: keep ACT free, evict on DVE.
                # qh=1 is the tail (ACT idle): alternate engines.
                if qh == 1 and ob_state["i"] % 2 == 0:
                    nc.scalar.copy(ob[:], po[:])
                else:
                    nc.vector.tensor_copy(ob[:], po[:])
                ob_state["i"] += 1
                qb = qh * (QH // 128) + i
                nc.sync.dma_start(out.ap()[qb * 128 : (qb + 1) * 128, :], ob[:])

            def emit_vproj(kb):
                ps = psM.tile([128, 256], F32, tag="av", name="ps_v", padded_shape=[128, DIM])
                for kc in range(NKC):
                    nc.tensor.matmul(
                        ps[:],
                        lhsT=xe_sb[:, kc, kb * 128 : (kb + 1) * 128],
                        rhs=wv_sb[:, kc, :],
                        start=(kc == 0),
                        stop=(kc == NKC - 1),
                    )
                nc.vector.tensor_copy(
                    v_sb[:, kb, :, 0:64], ps.rearrange("p (j c) -> p j c", c=64)
                )

            # Head-sequential segments: one head's 16-kb loop at a time, so
            # only one live av tile; the AV-drain + normalize of a segment
            # spill into the next segment's loop as `pending` thunks (one per
            # kb slot), keeping both PE and ACT streams gapless. The unpaired
            # score stream also keeps the PE dense (HAM stays warm).
            segs = [
                (qh, pr, hh) for qh in range(2) for pr in range(2) for hh in range(2)
            ]
            pending = []

            def make_segment(si, qh, pr, hh):
                qsl = slice(qh * QH, (qh + 1) * QH)
                j = 2 * pr + hh
                lag = 8 if si in (0, 5) else 2
                qt_j = qt_sb[pr][hh * 64 : hh * 64 + 64, qsl]
                kt_j = kt_sb[pr][hh * 64 : hh * 64 + 64, :]
                ss_hist = {}
                p_hist = {}
                avs_ref = {}

                def emit_scores(kb):
                    s = psM.tile([128, QH], F32, tag="s", name="s")
                    for nb in range(2):
                        nc.tensor.matmul(
                            s[:, nb * 512 : (nb + 1) * 512],
                            lhsT=kt_j[:, kb * 128 : (kb + 1) * 128],
                            rhs=qt_j[:, nb * 512 : (nb + 1) * 512],
                            start=True,
                            stop=True,
                        )
                    ss_hist[kb] = s

                def emit_pexp(kb):
                    s = ss_hist.pop(kb)
                    p2 = p2pool.tile([128, QH], BF, tag="p2", name="p2")
                    nc.scalar.activation(p2[:], s[:], Exp)
                    nc.vector.tensor_mul(p2[:], p2[:], m_sb[:, kb, qsl])
                    p_hist[kb] = p2

                def emit_av_alloc():
                    avs_ref["t"] = psM.tile([128, QH], F32, tag="av", name="av")

                def emit_av(kb):
                    p2 = p_hist.pop(kb)
                    av = avs_ref["t"]
                    for nb in range(2):
                        nc.tensor.matmul(
                            av[0:65, nb * 512 : (nb + 1) * 512],
                            lhsT=v_sb[:, kb, j, :],
                            rhs=p2[:, nb * 512 : (nb + 1) * 512],
                            start=(kb == 0),
                            stop=(kb == KB - 1),
                        )

                def emit_normalize():
                    # den -> 1/den -> broadcast -> att (DVE + gpsimd only)
                    av = avs_ref["t"]
                    dsl = slice(hh * QH, (hh + 1) * QH)
                    nc.vector.tensor_copy(den_sb[0:1, dsl], av[64:65, :])
                    nc.vector.reciprocal_approx_fast(rec_sb[0:1, dsl], den_sb[0:1, dsl])
                    rb = rbpool.tile([64, QH], F32, tag="rb", name="rb")
                    nc.gpsimd.partition_broadcast(rb[:], rec_sb[0:1, dsl])
                    nc.vector.scalar_tensor_tensor(
                        out=att_q[(pr, qh)][hh * 64 : hh * 64 + 64, :],
                        in0=av[0:64, :],
                        scalar=1.0,
                        in1=rb[:],
                        op0=Mult,
                        op1=Mult,
                    )

                return emit_scores, emit_pexp, emit_av_alloc, emit_av, emit_normalize, lag

            for si, (qh, pr, hh) in enumerate(segs):
                e_scores, e_pexp, e_av_alloc, e_av, e_norm, lag = make_segment(
                    si, qh, pr, hh
                )
                for kb in range(KB):
                    e_scores(kb)
                    if si == 0 and kb < 8:
                        emit_vproj(2 * kb)
                        emit_vproj(2 * kb + 1)
                    if si == 5 and kb < 8:
                        emit_po(0, kb)
                    if pending:
                        pending.pop(0)()
                    if kb == lag:
                        e_av_alloc()
                    if kb >= 1:
                        e_pexp(kb - 1)
                    if kb >= lag:
                        e_av(kb - lag)
                assert not pending
                e_pexp(KB - 1)
                pending = [
                    (lambda r=r, f=e_av: f(r)) for r in range(KB - lag, KB)
                ] + [e_norm]

            # tail: flush the last segment's drain, then qh=1 output projection
            for f in pending:
                f()
            pending = []
            for i in range(QH // 128):
                emit_po(1, i)
        p2pool.release()


def make_core_inputs(dec_xs, enc_xs, Wq, Wkv, Wo, mask):
    """Host-side sharding: returns list of 8 in_maps."""
    dec_xs = np.asarray(dec_xs, dtype=np.float32)
    enc_xs = np.asarray(enc_xs, dtype=np.float32)
    Wq = np.asarray(Wq, dtype=np.float32)
    Wkv = np.asarray(Wkv, dtype=np.float32)
    Wo = np.asarray(Wo, dtype=np.float32)
    mask = np.asarray(mask)

    Wk = Wkv[:DIM]
    Wv = Wkv[DIM:]

    xds, xes, mts = [], [], []
    for b in range(B):
        xds.append(np.ascontiguousarray(dec_xs[b].T).astype(bf16))
        xes.append(np.ascontiguousarray(enc_xs[b].T).astype(bf16))
        mts.append(np.ascontiguousarray(mask[b].T).astype(bf16))

    in_maps = []
    for c in range(8):
        b, g = divmod(c, 4)
        # local att col (j*64 + dk) <- global feature dk*H + (4g + j)
        hsel = np.array(
            [dk * H + (4 * g + j) for j in range(HPC) for dk in range(DK)],
            dtype=np.int64,
        )

        def arrange(w2d):
            # [D_in, M] -> [128, D_in//128, M] partition-major chunks
            d, mcols = w2d.shape
            return np.ascontiguousarray(
                w2d.reshape(d // 128, 128, mcols).transpose(1, 0, 2)
            ).astype(bf16)

        wq_l = arrange((Wq[hsel, :] / np.sqrt(DK)).T)
        wk_l = arrange(Wk[hsel, :].T)
        wv_l = arrange(Wv[hsel, :].T)
        wo_l = arrange(Wo[:, hsel].T)
        in_maps.append(
            {
                "xd": xds[b],
                "xe": xes[b],
                "mt": mts[b],
                "wq": wq_l,
                "wk": wk_l,
                "wv": wv_l,
                "wo": wo_l,
            }
        )
    return in_maps


_NC = None


def _get_nc():
    global _NC
    if _NC is None:
        _NC = build_program()
    return _NC


def kernel(dec_xs, enc_xs, Wq, Wkv, Wo, mask):
    nc = _get_nc()
    in_maps = make_core_inputs(dec_xs, enc_xs, Wq, Wkv, Wo, mask)
    res = run_bass_kernel_spmd(nc, in_maps, list(range(8)))
    out = np.zeros((B, S, DIM), np.float32)
    for c in range(8):
        out[c // 4] += np.asarray(res.results[c]["out"], dtype=np.float32)
    return out


# revision 16
# speedup vs baseline: 1.1592x; 1.1592x over previous
"""Cross-attention MHA (B=2, S=2048, DIM=1024, H=16, DK=64) on 8 trn2 cores.

Sharding: core c -> batch b = c//4, head group g = c%4 (heads 4g..4g+3).
Each core computes its 4 heads' attention + output-projection partial
[S, DIM] in bf16; host sums the 4 partials per batch.

v2 design (vs v1 phase-serial):
  The scalar engine's exp stream (~147us: 128 tiles of [128,1024], 1
  elem/cycle/lane @1.2GHz) is the hard bottleneck; the PE's matmul work
  (~125us warm) must ride inside that envelope instead of serializing
  with it. One continuous emission stream:
    prologue: Q proj -> K proj (evictions on ACT, DMA-paced)
    4 attention segments (qh, pr) of 16 k-blocks each, software-pipelined
      scores (PE, auto row-tiled pairs) -> exp (ACT) -> mask-mul (DVE)
      -> AV accumulate (PE)
    V-projection matmuls interleaved into segment 0 (PE slack),
    output projection for qh=0 interleaved into segment 2,
    normalize fused into AV eviction: den copy + reciprocal_approx_fast
      + gpsimd partition_broadcast + one scalar_tensor_tensor per head.
  PSUM: scores tag 4 banks + av/vproj/po shared tag 4 banks.
  Mask DMAs go through the gpsimd ring so they don't queue behind x/w
  loads on the sync ring. Output is bf16 (host accumulates in f32).

Device layout (per core), transposed so no on-chip transposes needed:
  xd = dec_xs[b].T     [DIM, S]   bf16
  xe = enc_xs[b].T     [DIM, S]   bf16
  mt = mask[b].T       [S_k, S_q] bf16   (0.0 / 1.0)
  wq/wk/wv [128, DIM/128, 256] bf16  cols (head_local, dk); wq pre-scaled
  wo [128, 256/128, DIM] bf16  rows (head_local, dk)
"""

import sys

sys.path.insert(0, "/opt/trn_rl_repo")

import numpy as np
import ml_dtypes

import concourse.bass as bass
import concourse.mybir as mybir
import concourse.tile as tile
from concourse import bacc
from concourse.bass_utils import run_bass_kernel_spmd

B, S, DIM, H, DK = 2, 2048, 1024, 16, 64
HPC = 4  # heads per core
AD = HPC * DK  # 256 local attention dims
KB = S // 128  # 16 k-blocks
QH = 1024  # q half
NKC = DIM // 128  # 8 contraction chunks
BF = mybir.dt.bfloat16
F32 = mybir.dt.float32
bf16 = ml_dtypes.bfloat16

P2_BUFS = 18  # p-tile ring depth (covers full-spill segments)


def build_program():
    nc = bacc.Bacc("TRN2", target_bir_lowering=False, debug=False, num_devices=8)

    xd = nc.dram_tensor("xd", [DIM, S], BF, kind="ExternalInput")
    xe = nc.dram_tensor("xe", [DIM, S], BF, kind="ExternalInput")
    mt = nc.dram_tensor("mt", [S, S], BF, kind="ExternalInput")
    wq = nc.dram_tensor("wq", [128, NKC, AD], BF, kind="ExternalInput")
    wk = nc.dram_tensor("wk", [128, NKC, AD], BF, kind="ExternalInput")
    wv = nc.dram_tensor("wv", [128, NKC, AD], BF, kind="ExternalInput")
    wo = nc.dram_tensor("wo", [128, AD // 128, DIM], BF, kind="ExternalInput")
    out = nc.dram_tensor("out", [S, DIM], BF, kind="ExternalOutput")

    with tile.TileContext(nc) as tc:
        build_tiles(tc, nc, xd, xe, mt, wq, wk, wv, wo, out)

    nc.compile()
    return nc


def build_tiles(tc, nc, xd, xe, mt, wq, wk, wv, wo, out):
    from contextlib import ExitStack

    Exp = mybir.ActivationFunctionType.Exp
    Mult = mybir.AluOpType.mult

    with ExitStack() as ctx:
        wpool = ctx.enter_context(tc.tile_pool(name="w", bufs=1))
        qkpool = ctx.enter_context(tc.tile_pool(name="qk", bufs=1))
        vpool = ctx.enter_context(tc.tile_pool(name="v", bufs=1))
        attpool = ctx.enter_context(tc.tile_pool(name="att", bufs=1))
        mpool = ctx.enter_context(tc.tile_pool(name="m", bufs=1))
        xepool = ctx.enter_context(tc.tile_pool(name="xe", bufs=1))
        denpool = ctx.enter_context(tc.tile_pool(name="den", bufs=1))
        recpool = ctx.enter_context(tc.tile_pool(name="rec", bufs=1))
        rbpool = ctx.enter_context(tc.tile_pool(name="rb", bufs=1))
        obpool = ctx.enter_context(tc.tile_pool(name="ob", bufs=2))

        # ---- long-lived SBUF ----
        wq_sb = wpool.tile([128, NKC, AD], BF, tag="wq", name="wq_sb")
        wk_sb = wpool.tile([128, NKC, AD], BF, tag="wk", name="wk_sb")
        wv_sb = wpool.tile([128, NKC, AD], BF, tag="wv", name="wv_sb")
        wo_sb = wpool.tile([128, AD // 128, DIM], BF, tag="wo", name="wo_sb")
        qt_sb = [qkpool.tile([128, S], BF, tag=f"qt{m}", name=f"qt{m}") for m in range(2)]
        kt_sb = [qkpool.tile([128, S], BF, tag=f"kt{m}", name=f"kt{m}") for m in range(2)]
        # V with a ones column at index 64 (row 64 of AV psum = softmax denom)
        v_sb = vpool.tile([128, KB, HPC, 65], BF, tag="v", name="v_sb")
        att_q = {
            (pr, qh): attpool.tile([128, QH], BF, tag=f"att{pr}{qh}", name=f"att{pr}{qh}")
            for pr in range(2)
            for qh in range(2)
        }
        m_sb = mpool.tile([128, KB, S], BF, tag="mask", name="m_sb")
        xe_sb = xepool.tile([128, NKC, S], BF, tag="xe", name="xe_sb")
        den_sb = denpool.tile([1, 2 * QH], F32, tag="den", name="den_sb")
        rec_sb = recpool.tile([1, 2 * QH], F32, tag="rec", name="rec_sb")

        # ---- prologue DMAs: xd chunks + wq first (Q proj gates everything) ----
        # xd pool opened/closed around Q proj so its space can be reused later
        xts = {}
        xdpool = tc.alloc_tile_pool(name="xd", bufs=2)
        for kc in range(2):
            xt = xdpool.tile([128, S], BF, tag="xd", name="xd_t")
            nc.sync.dma_start(xt[:], xd.ap()[kc * 128 : (kc + 1) * 128, :])
            xts[kc] = xt
        nc.sync.dma_start(wq_sb[:], wq.ap())

        # warm the ACT exp table off the critical path
        nc.scalar.activation(den_sb[0:1, 0:16], den_sb[0:1, 16:32], Exp)

        # ---- Q projection (kc-outer, xd prefetch, xe DMA interleave) ----
        with tc.tile_pool(name="psQ", bufs=4, space="PSUM") as psQ:
            ps_q = [psQ.tile([128, QH], F32, tag="ps_q", name=f"psq{i}") for i in range(4)]
            for kc in range(NKC):
                xt = xts.pop(kc)
                if kc + 2 < NKC:
                    nxt = xdpool.tile([128, S], BF, tag="xd", name="xd_t")
                    nc.sync.dma_start(nxt[:], xd.ap()[(kc + 2) * 128 : (kc + 3) * 128, :])
                    xts[kc + 2] = nxt
                for m in range(2):
                    for qq in range(2):
                        for nb in range(2):
                            nc.tensor.matmul(
                                ps_q[m * 2 + qq][:, nb * 512 : (nb + 1) * 512],
                                lhsT=wq_sb[:, kc, m * 128 : (m + 1) * 128],
                                rhs=xt[:, qq * QH + nb * 512 : qq * QH + (nb + 1) * 512],
                                start=(kc == 0),
                                stop=(kc == NKC - 1),
                            )
                nc.sync.dma_start(xe_sb[:, kc, :], xe.ap()[kc * 128 : (kc + 1) * 128, :])
            nc.sync.dma_start(wk_sb[:], wk.ap())
            nc.sync.dma_start(wv_sb[:], wv.ap())
            # mask DMAs on the sync ring AFTER xe/wk/wv: the queue order keeps
            # their 8MB of HBM traffic behind the loads that gate K proj
            for kb in range(KB):
                nc.sync.dma_start(m_sb[:, kb, :], mt.ap()[kb * 128 : (kb + 1) * 128, :])
            # evictions on ACT (idle in prologue); (m=0,qq=0) first: gates seg 0
            for m in range(2):
                for qq in range(2):
                    nc.scalar.copy(qt_sb[m][:, qq * QH : (qq + 1) * QH], ps_q[m * 2 + qq][:])
        xdpool.release()
        nc.sync.dma_start(wo_sb[:], wo.ap())

        # ---- K projection (m-sequential: kt[0] evicts ~7us earlier, which
        # gates the first attention segment) ----
        with tc.tile_pool(name="psK", bufs=4, space="PSUM") as psK:
            for m in range(2):
                ps_k = [
                    psK.tile([128, QH], F32, tag="ps_k", name=f"psk{i}") for i in range(2)
                ]
                for kc in range(NKC):
                    for qq in range(2):
                        for nb in range(2):
                            nc.tensor.matmul(
                                ps_k[qq][:, nb * 512 : (nb + 1) * 512],
                                lhsT=wk_sb[:, kc, m * 128 : (m + 1) * 128],
                                rhs=xe_sb[:, kc, qq * QH + nb * 512 : qq * QH + (nb + 1) * 512],
                                start=(kc == 0),
                                stop=(kc == NKC - 1),
                            )
                for qq in range(2):
                    nc.scalar.copy(kt_sb[m][:, qq * QH : (qq + 1) * QH], ps_k[qq][:])

        # ones for the denominator column (before first AV accumulate)
        nc.vector.memset(v_sb[:, :, :, 64:65], 1.0)

        # ---- main stream: attention segments + interleaved vproj / oproj ----
        p2pool = tc.alloc_tile_pool(name="p2", bufs=P2_BUFS)
        with tc.tile_pool(name="psM", bufs=2, space="PSUM") as psM:

            ob_state = {"i": 0}

            def emit_po(qh, i):
                # output projection for q-block i of half qh (128 rows)
                po = psM.tile([128, DIM], F32, tag="av", name="po")
                for cc in range(2):
                    for nb in range(2):
                        nc.tensor.matmul(
                            po[:, nb * 512 : (nb + 1) * 512],
                            lhsT=att_q[(cc, qh)][:, i * 128 : (i + 1) * 128],
                            rhs=wo_sb[:, cc, nb * 512 : (nb + 1) * 512],
                            start=(cc == 0),
                            stop=(cc == 1),
                        )
                ob = obpool.tile([128, DIM], BF, tag="ob", name="ob")
                # qh=0 runs inside the exp stretch: keep ACT free, evict on DVE.
                # qh=1 is the tail (ACT idle): alternate engines.
                if qh == 1 and ob_state["i"] % 2 == 0:
                    nc.scalar.copy(ob[:], po[:])
                else:
                    nc.vector.tensor_copy(ob[:], po[:])
                ob_state["i"] += 1
                qb = qh * (QH // 128) + i
                nc.sync.dma_start(out.ap()[qb * 128 : (qb + 1) * 128, :], ob[:])

            def emit_vproj(kb):
                ps = psM.tile([128, 256], F32, tag="av", name="ps_v", padded_shape=[128, DIM])
                for kc in range(NKC):
                    nc.tensor.matmul(
                        ps[:],
                        lhsT=xe_sb[:, kc, kb * 128 : (kb + 1) * 128],
                        rhs=wv_sb[:, kc, :],
                        start=(kc == 0),
                        stop=(kc == NKC - 1),
                    )
                nc.vector.tensor_copy(
                    v_sb[:, kb, :, 0:64], ps.rearrange("p (j c) -> p j c", c=64)
                )

            # Head-sequential segments: one head's 16-kb loop at a time, so
            # only one live av tile; the AV-drain + normalize of a segment
            # spill into the next segment's loop as `pending` thunks (one per
            # kb slot), keeping both PE and ACT streams gapless. The unpaired
            # score stream also keeps the PE dense (HAM stays warm).
            segs = [
                (qh, pr, hh) for qh in range(2) for pr in range(2) for hh in range(2)
            ]
            pending = []

            def make_segment(si, qh, pr, hh):
                qsl = slice(qh * QH, (qh + 1) * QH)
                j = 2 * pr + hh
                # si 0/5 host vproj/oproj in their kb slots: their own AV work
                # spills entirely into the next segment (lag = KB)
                lag = KB if si in (0, 5) else 2
                qt_j = qt_sb[pr][hh * 64 : hh * 64 + 64, qsl]
                kt_j = kt_sb[pr][hh * 64 : hh * 64 + 64, :]
                ss_hist = {}
                p_hist = {}
                avs_ref = {}

                def emit_scores(kb):
                    s = psM.tile([128, QH], F32, tag="s", name="s")
                    for nb in range(2):
                        nc.tensor.matmul(
                            s[:, nb * 512 : (nb + 1) * 512],
                            lhsT=kt_j[:, kb * 128 : (kb + 1) * 128],
                            rhs=qt_j[:, nb * 512 : (nb + 1) * 512],
                            start=True,
                            stop=True,
                        )
                    ss_hist[kb] = s

                def emit_pexp(kb):
                    s = ss_hist.pop(kb)
                    p2 = p2pool.tile([128, QH], BF, tag="p2", name="p2")
                    nc.scalar.activation(p2[:], s[:], Exp)
                    nc.vector.tensor_mul(p2[:], p2[:], m_sb[:, kb, qsl])
                    p_hist[kb] = p2

                def emit_av_alloc():
                    avs_ref["t"] = psM.tile([128, QH], F32, tag="av", name="av")

                def emit_av(kb):
                    p2 = p_hist.pop(kb)
                    av = avs_ref["t"]
                    for nb in range(2):
                        nc.tensor.matmul(
                            av[0:65, nb * 512 : (nb + 1) * 512],
                            lhsT=v_sb[:, kb, j, :],
                            rhs=p2[:, nb * 512 : (nb + 1) * 512],
                            start=(kb == 0),
                            stop=(kb == KB - 1),
                        )

                def emit_normalize():
                    # den -> 1/den -> broadcast -> att (DVE + gpsimd only)
                    av = avs_ref["t"]
                    dsl = slice(hh * QH, (hh + 1) * QH)
                    nc.vector.tensor_copy(den_sb[0:1, dsl], av[64:65, :])
                    nc.vector.reciprocal_approx_fast(rec_sb[0:1, dsl], den_sb[0:1, dsl])
                    rb = rbpool.tile([64, QH], F32, tag="rb", name="rb")
                    nc.gpsimd.partition_broadcast(rb[:], rec_sb[0:1, dsl])
                    nc.vector.scalar_tensor_tensor(
                        out=att_q[(pr, qh)][hh * 64 : hh * 64 + 64, :],
                        in0=av[0:64, :],
                        scalar=1.0,
                        in1=rb[:],
                        op0=Mult,
                        op1=Mult,
                    )

                return emit_scores, emit_pexp, emit_av_alloc, emit_av, emit_normalize, lag

            for si, (qh, pr, hh) in enumerate(segs):
                e_scores, e_pexp, e_av_alloc, e_av, e_norm, lag = make_segment(
                    si, qh, pr, hh
                )
                for kb in range(KB):
                    e_scores(kb)
                    if si == 0:
                        emit_vproj(kb)
                    if si == 5 and kb % 2 == 1:
                        emit_po(0, kb // 2)
                    if pending:
                        pending.pop(0)()
                    if kb == lag:
                        e_av_alloc()
                    if kb >= 1:
                        e_pexp(kb - 1)
                    if kb >= lag:
                        e_av(kb - lag)
                while pending:
                    pending.pop(0)()
                e_pexp(KB - 1)
                drain = []
                for idx, r in enumerate(range(KB - lag, KB)):
                    if idx == 0 and lag == KB:
                        # spill segment: av tile allocated by the first thunk
                        drain.append(
                            lambda r=r, fa=e_av_alloc, f=e_av: (fa(), f(r))[-1]
                        )
                    else:
                        drain.append(lambda r=r, f=e_av: f(r))
                pending = drain + [e_norm]

            # tail: flush the last segment's drain, then qh=1 output projection
            while pending:
                pending.pop(0)()
            for i in range(QH // 128):
                emit_po(1, i)
        p2pool.release()


def make_core_inputs(dec_xs, enc_xs, Wq, Wkv, Wo, mask):
    """Host-side sharding: returns list of 8 in_maps."""
    dec_xs = np.asarray(dec_xs, dtype=np.float32)
    enc_xs = np.asarray(enc_xs, dtype=np.float32)
    Wq = np.asarray(Wq, dtype=np.float32)
    Wkv = np.asarray(Wkv, dtype=np.float32)
    Wo = np.asarray(Wo, dtype=np.float32)
    mask = np.asarray(mask)

    Wk = Wkv[:DIM]
    Wv = Wkv[DIM:]

    xds, xes, mts = [], [], []
    for b in range(B):
        xds.append(np.ascontiguousarray(dec_xs[b].T).astype(bf16))
        xes.append(np.ascontiguousarray(enc_xs[b].T).astype(bf16))
        mts.append(np.ascontiguousarray(mask[b].T).astype(bf16))

    in_maps = []
    for c in range(8):
        b, g = divmod(c, 4)
        # local att col (j*64 + dk) <- global feature dk*H + (4g + j)
        hsel = np.array(
            [dk * H + (4 * g + j) for j in range(HPC) for dk in range(DK)],
            dtype=np.int64,
        )

        def arrange(w2d):
            # [D_in, M] -> [128, D_in//128, M] partition-major chunks
            d, mcols = w2d.shape
            return np.ascontiguousarray(
                w2d.reshape(d // 128, 128, mcols).transpose(1, 0, 2)
            ).astype(bf16)

        wq_l = arrange((Wq[hsel, :] / np.sqrt(DK)).T)
        wk_l = arrange(Wk[hsel, :].T)
        wv_l = arrange(Wv[hsel, :].T)
        wo_l = arrange(Wo[:, hsel].T)
        in_maps.append(
            {
                "xd": xds[b],
                "xe": xes[b],
                "mt": mts[b],
                "wq": wq_l,
                "wk": wk_l,
                "wv": wv_l,
                "wo": wo_l,
            }
        )
    return in_maps


_NC = None


def _get_nc():
    global _NC
    if _NC is None:
        _NC = build_program()
    return _NC


def kernel(dec_xs, enc_xs, Wq, Wkv, Wo, mask):
    nc = _get_nc()
    in_maps = make_core_inputs(dec_xs, enc_xs, Wq, Wkv, Wo, mask)
    res = run_bass_kernel_spmd(nc, in_maps, list(range(8)))
    out = np.zeros((B, S, DIM), np.float32)
    for c in range(8):
        out[c // 4] += np.asarray(res.results[c]["out"], dtype=np.float32)
    return out
